# revision 10
# baseline (speedup 1.0000x reference)
"""Distributed CL loss kernel for Trainium2 (8 NeuronCores).

Reference computes  mean_i sum_j ||s_i - t_j||^2 * [tg_i == tg_j] / cnt[tg_i]
with the [N, N] pairwise-distance matrix.  Because the mask only depends on
the class labels, the whole loss collapses to per-class aggregates:

  loss = (1/N) * [ sum|s|^2 + sum|t|^2 - 2 * sum_c S_c.T_c / cnt_c ]

with S_c / T_c the class-sums of fm_s / fm_t rows.  Device work per core
(rows sharded 512 s-rows + 512 t-rows, fp8e4m3, one-hot cols appended).

Measured realities this schedule is built around (from NTFF traces):
  * the input stream drains at ~200-250 GB/s aggregate with all 8 cores
    up (HBM loaded latency), independent of queue count / descriptor
    shape — so the 1.04 MB shard takes ~5 us plus ~2 us lead-in, and the
    only free variables are compute pacing and the post-stream tail,
  * ACT ACTIVATE = (352+FD)/1.2GHz, DVE STT = ~(58+1.2*FD)/0.96GHz,
    PSUM evacuation runs at 1x in every dtype — so the last-arriving
    tile's square and the final PSUM evac are split in half across both
    engines to halve the tail,
  * the ACT activation-table load (~1.3 us) is pulled to t=0 by a 1-elem
    dummy activation issued before any semaphore wait,
  * the NEFF window carries ~1.3 us bass-preamble tail + ~8 us of
    load-time injected postamble (254 serial semaphore resets) that no
    kernel change can touch.

Schedule: per-tile DMAs (sync HWDGE ring: s-tiles 0-3; gpsimd SWDGE:
t-tiles 4-7), per-tile fused square+accumulate ops (ACT: 0,1,2 / DVE:
4,5,6 / tiles 3 and 7 as half-tile ops on both engines), DoubleRow fp8
class-sum matmuls per tile-pair in arrival order, S|T evacuated as
engine-parallel half copies, one combined [16,2048] bf16 output DMA
(sync) + one [128,10] f32 accum DMA (gpsimd).  Output-DMA receipts hide
inside the postamble's per-engine DRAIN + quiesce.

fp8 notes: e4m3 quantization biases sum|x|^2 by ~+0.1% and the cross term
contributes only ~0.01% of the loss; measured end-to-end relative error
~7e-4, well inside the 2e-2 gate.  All accumulators (PSUM, accum_out) are
fp32.
"""

import numpy as np

N, D, NUM_CLASSES = 4096, 1024, 10
NCORES = 8
RPC = N // NCORES   # rows per core (both fm_s and fm_t are row-sharded)
KT = RPC // 128     # 128-row k-tiles per core per tensor (4)
W = 2 * KT          # total k-tiles per core (s then t) = 8
CP = 16             # class dim padded for alignment
DW = D + CP         # tile width: data + appended one-hot columns
NSQ = 10            # square ops (6 full-tile + 4 half-tile)

_STATE = {}
LAST_RUN = None  # BassKernelResults of the most recent device run (for test.py)


def build_nc_raw():
    import concourse.bacc as bacc
    import concourse.mybir as mybir

    f32 = mybir.dt.float32
    f8 = mybir.dt.float8e4
    bf16 = mybir.dt.bfloat16
    nc = bacc.Bacc(
        "TRN2",
        target_bir_lowering=False,
        debug=False,
        enable_asserts=False,
        num_devices=NCORES,
        # this kernel never reads the partition id (cores differ only by
        # their input slices) and uses no monotonic semaphores — dropping
        # both trims the framework preamble
        enable_partition_id=False,
        monotonic_sem_count=0,
    )

    # tile-major DRAM layout: tile w is one contiguous 133 KB block
    x_in = nc.dram_tensor("x_in", (W, 128, DW), f8, kind="ExternalInput")
    sq_out = nc.dram_tensor("sq_out", (128, NSQ), f32, kind="ExternalOutput")
    ST_out = nc.dram_tensor("ST_out", (CP, 2 * D), bf16, kind="ExternalOutput")

    x_sb = nc.alloc_sbuf_tensor("x_sb", [128, W, DW], f8)
    ST_sb = nc.alloc_sbuf_tensor("ST_sb", [CP, 2 * D], bf16)
    stats = nc.alloc_sbuf_tensor("stats", [128, NSQ], f32)

    pS = nc.alloc_psum_tensor("pS", [CP, D], f32)  # 2 banks
    pT = nc.alloc_psum_tensor("pT", [CP, D], f32)  # 2 banks
    # fp8 square scratch: keeps the scratch WRITES small so they do not
    # stall the input-DMA SBUF writes; slot 0 = ACT, slot 1 = DVE (reused
    # serially per engine — engine program order makes that safe).  Only
    # the f32 accum_out feeds the result, the scratch value is never read.
    sq_scr = nc.alloc_sbuf_tensor("sq_scr", [128, 2, D], f8)

    k_sems = [nc.alloc_semaphore(f"k_sem{w}") for w in range(W)]
    sS = nc.alloc_semaphore("sS")
    sT = nc.alloc_semaphore("sT")
    ev = nc.alloc_semaphore("ev")
    sq_done = nc.alloc_semaphore("sq_done")
    out_sem = nc.alloc_semaphore("out_sem")

    Sq = mybir.ActivationFunctionType.Square
    Copy = mybir.ActivationFunctionType.Copy
    ADD = mybir.AluOpType.add
    MUL = mybir.AluOpType.mult
    DR = mybir.MatmulPerfMode.DoubleRow

    xs = x_sb.ap()

    def square(engine, w, half, slot):
        # fused square + free-axis accumulate: stats[:, slot] = sum_d x^2
        # over (a half of) tile w's 1024 data columns (one-hot tail
        # excluded).  half: None = full tile, 0/1 = 512-col half.
        lo, hi = (0, D) if half is None else (512 * half, 512 * (half + 1))
        scr = 0 if engine is nc.scalar else 1
        src = xs[:, w, lo:hi]
        if engine is nc.scalar:
            op = engine.activation(
                sq_scr.ap()[:, scr, 0 : hi - lo],
                src,
                Sq,
                accum_out=stats.ap()[:, slot : slot + 1],
            )
        else:
            op = engine.scalar_tensor_tensor(
                sq_scr.ap()[:, scr, 0 : hi - lo],
                src,
                0.0,
                src,
                ADD,
                MUL,
                accum_out=stats.ap()[:, slot : slot + 1],
            )
        op.then_inc(sq_done, 1)

    def evac_half(engine, dst_lo, src_psum, src_lo):
        # PSUM -> SBUF bf16, one 512-col half (runs at 1x in any dtype,
        # so halves on both engines in parallel halve the latency)
        dst = ST_sb.ap()[:, dst_lo : dst_lo + 512]
        src = src_psum.ap()[:, src_lo : src_lo + 512]
        if engine is nc.scalar:
            op = engine.activation(dst, src, Copy)
        else:
            op = engine.tensor_copy(dst, src)
        op.then_inc(ev, 1)

    with nc.Block() as block:

        @block.sync
        def _(sync):
            for w in (0, 1, 2, 3):
                sync.dma_start(xs[:, w, :], x_in.ap()[w, :, :]).then_inc(
                    k_sems[w], 16
                )
            # combined S|T output once all four half-evacuations landed
            sync.wait_ge(ev, 4)
            sync.dma_start(ST_out.ap(), ST_sb.ap()).then_inc(out_sem, 16)
            # no explicit out_sem wait: the NEFF postamble's per-engine
            # DRAIN + runtime pending-DMA quiesce already order the output
            # DMAs before execution-complete
            sync.wait_ge(out_sem, 0)

        @block.scalar
        def _(scalar):
            # 1-element dummy activation BEFORE any wait: pulls the
            # auto-inserted ACT table load to t=0, under the input stream
            scalar.activation(sq_scr.ap()[:, 0, 0:1], stats.ap()[:, 0:1], Sq)
            for w in (0, 1, 2):
                scalar.wait_ge(k_sems[w], 16)
                square(scalar, w, None, w)
            scalar.wait_ge(k_sems[3], 16)
            square(scalar, 3, 0, 3)           # half A of tile 3
            scalar.wait_ge(sS, 2)
            evac_half(scalar, 0, pS, 0)       # S cols 0:512
            scalar.wait_ge(k_sems[7], 16)
            square(scalar, 7, 0, 7)           # half A of tile 7
            scalar.wait_ge(sT, 2)
            evac_half(scalar, D, pT, 0)       # T cols 0:512

        @block.vector
        def _(vector):
            for w in (4, 5, 6):
                vector.wait_ge(k_sems[w], 16)
                square(vector, w, None, w)
            vector.wait_ge(k_sems[3], 16)
            square(vector, 3, 1, 8)           # half B of tile 3
            vector.wait_ge(sS, 2)
            evac_half(vector, 512, pS, 512)   # S cols 512:1024
            vector.wait_ge(k_sems[7], 16)
            square(vector, 7, 1, 9)           # half B of tile 7
            vector.wait_ge(sT, 2)
            evac_half(vector, D + 512, pT, 512)  # T cols 512:1024

        @block.gpsimd
        def _(gpsimd):
            for w in (4, 5, 6, 7):
                gpsimd.dma_start(xs[:, w, :], x_in.ap()[w, :, :]).then_inc(
                    k_sems[w], 16
                )
            gpsimd.wait_ge(sq_done, NSQ)
            gpsimd.dma_start(sq_out.ap(), stats.ap()).then_inc(out_sem, 16)

        @block.tensor
        def _(tensor):
            # DoubleRow fp8: each matmul contracts a PAIR of 128-row k-tiles
            # (AP dim1 = pair index).  Accumulation groups per PSUM bank run
            # start-pair -> stop-pair; pairs interleave S/T by arrival.
            def mm(bank, dsem, pair, start, stop):
                a = 2 * pair
                lhsT = xs[:, a : a + 2, D:DW]
                for h in range(2):
                    m = tensor.matmul(
                        bank.ap()[:, 512 * h : 512 * (h + 1)],
                        lhsT,
                        xs[:, a : a + 2, 512 * h : 512 * (h + 1)],
                        start=start,
                        stop=stop,
                        perf_mode=DR,
                    )
                    if stop:
                        m.then_inc(dsem, 1)

            tensor.wait_ge(k_sems[0], 16)
            tensor.wait_ge(k_sems[1], 16)
            mm(pS, sS, 0, True, False)        # s pair (0,1)
            tensor.wait_ge(k_sems[4], 16)
            tensor.wait_ge(k_sems[5], 16)
            mm(pT, sT, 2, True, False)        # t pair (4,5)
            tensor.wait_ge(k_sems[2], 16)
            tensor.wait_ge(k_sems[3], 16)
            mm(pS, sS, 1, False, True)        # s pair (2,3) -> closes S
            tensor.wait_ge(k_sems[6], 16)
            tensor.wait_ge(k_sems[7], 16)
            mm(pT, sT, 3, False, True)        # t pair (6,7) -> closes T

    nc.compile()
    return nc


def _get_nc():
    if "nc" not in _STATE:
        _STATE["nc"] = build_nc_raw()
    return _STATE["nc"]


def kernel(fm_s, fm_t, targets, fusion_true=0, **_unused):
    global LAST_RUN
    import ml_dtypes
    from concourse.bass_utils import run_bass_kernel_spmd

    f8 = ml_dtypes.float8_e4m3
    fm_s = np.ascontiguousarray(np.asarray(fm_s, dtype=np.float32))
    fm_t = np.ascontiguousarray(np.asarray(fm_t, dtype=np.float32))
    tg = np.asarray(targets).astype(np.int64).ravel()
    assert fm_s.shape == (N, D) and fm_t.shape == (N, D) and tg.shape == (N,)

    oh = (tg[:, None] == np.arange(CP, dtype=np.int64)[None, :]).astype(np.float32)
    counts = np.bincount(tg, minlength=CP).astype(np.float64)[:CP]
    # append the one-hot columns to every row so each 128-row k-tile is
    # self-contained (the PE takes lhsT from the tile's own tail columns)
    s_aug = np.concatenate([fm_s, oh], axis=1).astype(f8)
    t_aug = np.concatenate([fm_t, oh], axis=1).astype(f8)

    in_maps = []
    for c in range(NCORES):
        # tile-major: tile w = rows (128w .. 128w+127) of this core's
        # shard (s tiles 0-3 then t tiles 4-7), one contiguous block
        s_c = s_aug[c * RPC : (c + 1) * RPC].reshape(KT, 128, DW)
        t_c = t_aug[c * RPC : (c + 1) * RPC].reshape(KT, 128, DW)
        x = np.ascontiguousarray(np.concatenate([s_c, t_c], axis=0))
        in_maps.append({"x_in": x})

    nc = _get_nc()
    LAST_RUN = run_bass_kernel_spmd(nc, in_maps, list(range(NCORES)))
    res = LAST_RUN.results

    ss_tt = 0.0
    S = np.zeros((CP, D), np.float64)
    T = np.zeros((CP, D), np.float64)
    for r in res:
        ss_tt += float(r["sq_out"].astype(np.float64).sum())
        ST = r["ST_out"].astype(np.float64)
        S += ST[:, 0:D]
        T += ST[:, D : 2 * D]

    safe = np.where(counts > 0, counts, 1.0)
    dot = float(((S * T).sum(axis=1) / safe).sum())
    loss = (ss_tt - 2.0 * dot) / N
    return np.array(loss, dtype=np.float32)


# revision 11
# speedup vs baseline: 1.1145x; 1.1145x over previous
"""Distributed CL loss kernel for Trainium2 (8 NeuronCores).

Reference computes  mean_i sum_j ||s_i - t_j||^2 * [tg_i == tg_j] / cnt[tg_i]
with the [N, N] pairwise-distance matrix.  Because the mask only depends on
the class labels, the whole loss collapses to per-class aggregates:

  loss = (1/N) * [ sum|s|^2 + sum|t|^2 - 2 * sum_c S_c.T_c / cnt_c ]

with S_c / T_c the class-sums of fm_s / fm_t rows.  Device work per core
(rows sharded 512 s-rows + 512 t-rows, fp8e4m3, one-hot cols appended).

Measured realities this schedule is built around (NTFF traces):
  * the input stream drains at ~200-250 GB/s aggregate with all 8 cores up
    (HBM loaded latency); 2080 B/partition descriptors (2-tile chunks)
    beat 1040 B ones, and extra queues don't raise the aggregate — so all
    four chunks ride the sync HWDGE ring FIFO, with a tiny warm-up DMA in
    front to absorb the ~1.2 us cold-pipeline cost of the first transfer,
  * ACT ACTIVATE = (352+FD)/1.2GHz +278 ns accumulator-read; DVE STT =
    ~(58+1.2*FD)/0.96GHz +81 ns read — per-tile squares interleave across
    both engines paced by chunk sems,
  * the ACT activation-table load (~1.3 us) auto-inserts directly before
    the first ACTIVATE; keeping a single sem-wait on that op keeps the
    load at t=0 (multi-waits become separate preceding instructions and
    drag the load behind them),
  * PSUM evacuation: S via ACT activation-Copy, T via DVE f32 tensor_copy
    (no dtype cast — the cast path measured 1x),
  * the NEFF window carries ~1.3 us bass-preamble tail and ~8 us of
    load-time injected postamble (254 serial semaphore resets across the
    engines) that no kernel change can touch; output-DMA receipts hide
    inside that postamble's per-engine DRAIN + quiesce.

fp8 notes: e4m3 quantization biases sum|x|^2 by ~+0.1% and the cross term
contributes only ~0.01% of the loss; measured end-to-end relative error
~7e-4, well inside the 2e-2 gate.  All accumulators (PSUM, accum_out) are
fp32.
"""

import numpy as np

N, D, NUM_CLASSES = 4096, 1024, 10
NCORES = 8
RPC = N // NCORES   # rows per core (both fm_s and fm_t are row-sharded)
KT = RPC // 128     # 128-row k-tiles per core per tensor (4)
W = 2 * KT          # total k-tiles per core (s then t) = 8
NCH = 4             # input chunks (2 tiles each)
CP = 16             # class dim padded for alignment
DW = D + CP         # tile width: data + appended one-hot columns

# square-op assignment: ACT takes even tiles, DVE odd — tile w arrives with
# chunk w//2, so both engines start after chunk 0 and stay chunk-paced,
# and every square carries exactly ONE sem wait.
SQ_ACT = [0, 2, 4, 6]
SQ_DVE = [1, 3, 5, 7]

_STATE = {}
LAST_RUN = None  # BassKernelResults of the most recent device run (for test.py)


def build_nc_raw():
    import concourse.bacc as bacc
    import concourse.mybir as mybir

    f32 = mybir.dt.float32
    f8 = mybir.dt.float8e4
    nc = bacc.Bacc(
        "TRN2",
        target_bir_lowering=False,
        debug=False,
        enable_asserts=False,
        num_devices=NCORES,
        # this kernel never reads the partition id (cores differ only by
        # their input slices) and uses no monotonic semaphores — dropping
        # both trims the framework preamble
        enable_partition_id=False,
        monotonic_sem_count=0,
    )

    # chunk-major DRAM layout: chunk i is one contiguous 266 KB block,
    # giving a single contiguous 2080 B M2S descriptor per partition
    x_in = nc.dram_tensor("x_in", (NCH, 128, 2 * DW), f8, kind="ExternalInput")
    sq_out = nc.dram_tensor("sq_out", (128, W), f32, kind="ExternalOutput")
    ST_out = nc.dram_tensor("ST_out", (CP, 2 * D), f32, kind="ExternalOutput")

    x_sb = nc.alloc_sbuf_tensor("x_sb", [128, W, DW], f8)
    ST_sb = nc.alloc_sbuf_tensor("ST_sb", [CP, 2 * D], f32)
    stats = nc.alloc_sbuf_tensor("stats", [128, W], f32)
    warm_sb = nc.alloc_sbuf_tensor("warm_sb", [128, 32], f8)

    pS = nc.alloc_psum_tensor("pS", [CP, D], f32)  # 2 banks
    pT = nc.alloc_psum_tensor("pT", [CP, D], f32)  # 2 banks
    # fp8 square scratch: keeps the scratch WRITES small (1 KB/partition/op)
    # so they do not stall the input-DMA SBUF writes; slot 0 = ACT, slot 1 =
    # DVE, reused serially per engine.  Only the f32 accum_out feeds the
    # result, the scratch value is never read.
    sq_scr = nc.alloc_sbuf_tensor("sq_scr", [128, 2, D], f8)

    k_sems = [nc.alloc_semaphore(f"k_sem{i}") for i in range(NCH)]
    warm_sem = nc.alloc_semaphore("warm_sem")
    sS = nc.alloc_semaphore("sS")
    sT = nc.alloc_semaphore("sT")
    ev = nc.alloc_semaphore("ev")
    sq_done = nc.alloc_semaphore("sq_done")
    out_sem = nc.alloc_semaphore("out_sem")

    Sq = mybir.ActivationFunctionType.Square
    Copy = mybir.ActivationFunctionType.Copy
    ADD = mybir.AluOpType.add
    MUL = mybir.AluOpType.mult
    DR = mybir.MatmulPerfMode.DoubleRow

    xs = x_sb.ap()

    def square(engine, w):
        # fused square + free-axis accumulate: stats[:, w] = sum_d x^2 over
        # tile w's 1024 data columns (one-hot tail excluded).
        if engine is nc.scalar:
            op = engine.activation(
                sq_scr.ap()[:, 0, :],
                xs[:, w, 0:D],
                Sq,
                accum_out=stats.ap()[:, w : w + 1],
            )
        else:
            op = engine.scalar_tensor_tensor(
                sq_scr.ap()[:, 1, :],
                xs[:, w, 0:D],
                0.0,
                xs[:, w, 0:D],
                ADD,
                MUL,
                accum_out=stats.ap()[:, w : w + 1],
            )
        op.then_inc(sq_done, 1)

    with nc.Block() as block:

        @block.sync
        def _(sync):
            # tiny warm-up read first: absorbs the cold-pipeline cost of
            # the ring's first transfer so chunk 0 lands earlier
            sync.dma_start(warm_sb.ap(), x_in.ap()[0, :, 0:32]).then_inc(
                warm_sem, 16
            )
            # all four input chunks back-to-back on one ring (FIFO drain,
            # all 16 SDMA engines); chunk i = tiles (2i, 2i+1) = DR pair i
            for i in range(NCH):
                sync.dma_start(
                    x_sb.ap()[:, 2 * i : 2 * i + 2, :],
                    x_in.ap()[i, :, :],
                ).then_inc(k_sems[i], 16)
            # combined S|T output once both evacuations landed
            sync.wait_ge(ev, 2)
            sync.dma_start(ST_out.ap(), ST_sb.ap()).then_inc(out_sem, 16)
            # no explicit out_sem wait: the NEFF postamble's per-engine
            # DRAIN + runtime pending-DMA quiesce already order the output
            # DMAs before execution-complete
            sync.wait_ge(out_sem, 0)

        @block.scalar
        def _(scalar):
            # ACT table load auto-inserts before the first ACTIVATE (the
            # single-wait square below), i.e. at t=0 under the stream
            for w in SQ_ACT:
                scalar.wait_ge(k_sems[w // 2], 16)
                square(scalar, w)
            scalar.wait_ge(sS, 2)
            scalar.activation(ST_sb.ap()[:, 0:D], pS.ap(), Copy).then_inc(ev, 1)

        @block.vector
        def _(vector):
            for w in SQ_DVE:
                vector.wait_ge(k_sems[w // 2], 16)
                square(vector, w)
            vector.wait_ge(sT, 2)
            # f32 -> f32, no cast: the dtype-converting copy measured 1x
            vector.tensor_copy(ST_sb.ap()[:, D : 2 * D], pT.ap()).then_inc(ev, 1)

        @block.gpsimd
        def _(gpsimd):
            # sq_out rides the SW-DGE queue, keeping the HWDGE ring free
            gpsimd.wait_ge(sq_done, W)
            gpsimd.dma_start(sq_out.ap(), stats.ap()).then_inc(out_sem, 16)

        @block.tensor
        def _(tensor):
            # DoubleRow fp8: each matmul contracts a PAIR of 128-row k-tiles
            # (AP dim1 = pair index).  Accumulation groups per PSUM bank run
            # pairA (start) -> pairB (stop); chunk i holds exactly pair i.
            def mm(bank, dsem, pair, start, stop):
                a = 2 * pair
                lhsT = xs[:, a : a + 2, D:DW]
                for h in range(2):
                    m = tensor.matmul(
                        bank.ap()[:, 512 * h : 512 * (h + 1)],
                        lhsT,
                        xs[:, a : a + 2, 512 * h : 512 * (h + 1)],
                        start=start,
                        stop=stop,
                        perf_mode=DR,
                    )
                    if stop:
                        m.then_inc(dsem, 1)

            tensor.wait_ge(k_sems[0], 16)
            mm(pS, sS, 0, True, False)
            tensor.wait_ge(k_sems[1], 16)
            mm(pS, sS, 1, False, True)
            tensor.wait_ge(k_sems[2], 16)
            mm(pT, sT, 2, True, False)
            tensor.wait_ge(k_sems[3], 16)
            mm(pT, sT, 3, False, True)

    nc.compile()
    return nc


def _get_nc():
    if "nc" not in _STATE:
        _STATE["nc"] = build_nc_raw()
    return _STATE["nc"]


def kernel(fm_s, fm_t, targets, fusion_true=0, **_unused):
    global LAST_RUN
    import ml_dtypes
    from concourse.bass_utils import run_bass_kernel_spmd

    f8 = ml_dtypes.float8_e4m3
    fm_s = np.ascontiguousarray(np.asarray(fm_s, dtype=np.float32))
    fm_t = np.ascontiguousarray(np.asarray(fm_t, dtype=np.float32))
    tg = np.asarray(targets).astype(np.int64).ravel()
    assert fm_s.shape == (N, D) and fm_t.shape == (N, D) and tg.shape == (N,)

    oh = (tg[:, None] == np.arange(CP, dtype=np.int64)[None, :]).astype(np.float32)
    counts = np.bincount(tg, minlength=CP).astype(np.float64)[:CP]
    # append the one-hot columns to every row so each 128-row k-tile is
    # self-contained (the PE takes lhsT from the tile's own tail columns)
    s_aug = np.concatenate([fm_s, oh], axis=1).astype(f8)
    t_aug = np.concatenate([fm_t, oh], axis=1).astype(f8)

    in_maps = []
    for c in range(NCORES):
        # chunk-major + partition-major: chunk i holds tiles (2i, 2i+1);
        # x[i, p, :] = rows (256i + p, 256i + 128 + p) of this core's shard
        # (s tiles 0-3 then t tiles 4-7), each a contiguous 2080 B read.
        s_c = s_aug[c * RPC : (c + 1) * RPC].reshape(KT, 128, DW)
        t_c = t_aug[c * RPC : (c + 1) * RPC].reshape(KT, 128, DW)
        tiles = np.concatenate([s_c, t_c], axis=0)          # [W, 128, DW]
        x = np.ascontiguousarray(
            tiles.reshape(NCH, 2, 128, DW)
            .transpose(0, 2, 1, 3)
            .reshape(NCH, 128, 2 * DW)
        )
        in_maps.append({"x_in": x})

    nc = _get_nc()
    LAST_RUN = run_bass_kernel_spmd(nc, in_maps, list(range(NCORES)))
    res = LAST_RUN.results

    ss_tt = 0.0
    S = np.zeros((CP, D), np.float64)
    T = np.zeros((CP, D), np.float64)
    for r in res:
        ss_tt += float(r["sq_out"].astype(np.float64).sum())
        ST = r["ST_out"].astype(np.float64)
        S += ST[:, 0:D]
        T += ST[:, D : 2 * D]

    safe = np.where(counts > 0, counts, 1.0)
    dot = float(((S * T).sum(axis=1) / safe).sum())
    loss = (ss_tt - 2.0 * dot) / N
    return np.array(loss, dtype=np.float32)


# revision 12
# speedup vs baseline: 1.1473x; 1.0294x over previous
"""Distributed CL loss kernel for Trainium2 (8 NeuronCores).

Reference computes  mean_i sum_j ||s_i - t_j||^2 * [tg_i == tg_j] / cnt[tg_i]
with the [N, N] pairwise-distance matrix.  Because the mask only depends on
the class labels, the whole loss collapses to per-class aggregates:

  loss = (1/N) * [ sum|s|^2 + sum|t|^2 - 2 * sum_c S_c.T_c / cnt_c ]

with S_c / T_c the class-sums of fm_s / fm_t rows.  Device work per core
(rows sharded 512 s-rows + 512 t-rows, fp8e4m3, one-hot cols appended).

Measured realities this schedule is built around (NTFF traces):
  * the input stream drains at ~200-250 GB/s aggregate with all 8 cores up
    (HBM loaded latency); 2080 B/partition descriptors (2-tile chunks)
    beat 1040 B ones, and extra queues don't raise the aggregate — so all
    four chunks ride the sync HWDGE ring FIFO, with a tiny warm-up DMA in
    front to absorb the ~1.2 us cold-pipeline cost of the first transfer,
  * ACT ACTIVATE = (352+FD)/1.2GHz +278 ns accumulator-read; DVE STT =
    ~(58+1.2*FD)/0.96GHz +81 ns read — per-tile squares interleave across
    both engines paced by chunk sems,
  * the ACT activation-table load (~1.3 us) auto-inserts directly before
    the first ACTIVATE; keeping a single sem-wait on that op keeps the
    load at t=0 (multi-waits become separate preceding instructions and
    drag the load behind them),
  * PSUM evacuation: S via ACT activation-Copy, T via DVE f32 tensor_copy
    (no dtype cast — the cast path measured 1x),
  * the NEFF window carries ~1.3 us bass-preamble tail and ~8 us of
    load-time injected postamble (254 serial semaphore resets across the
    engines) that no kernel change can touch; output-DMA receipts hide
    inside that postamble's per-engine DRAIN + quiesce.

fp8 notes: e4m3 quantization biases sum|x|^2 by ~+0.1% and the cross term
contributes only ~0.01% of the loss; measured end-to-end relative error
~7e-4, well inside the 2e-2 gate.  All accumulators (PSUM, accum_out) are
fp32.
"""

import numpy as np

N, D, NUM_CLASSES = 4096, 1024, 10
NCORES = 8
RPC = N // NCORES   # rows per core (both fm_s and fm_t are row-sharded)
KT = RPC // 128     # 128-row k-tiles per core per tensor (4)
W = 2 * KT          # total k-tiles per core (s then t) = 8
NCH = 4             # input chunks (2 tiles each)
CP = 16             # class dim padded for alignment
DW = D + CP         # tile width: data + appended one-hot columns

# square-op assignment: ACT takes even tiles, DVE odd — tile w arrives with
# chunk w//2, so both engines start after chunk 0 and stay chunk-paced,
# and every square carries exactly ONE sem wait.
SQ_ACT = [0, 2, 4, 6]
SQ_DVE = [1, 3, 5, 7]

_STATE = {}
LAST_RUN = None  # BassKernelResults of the most recent device run (for test.py)


def build_nc_raw():
    import concourse.bacc as bacc
    import concourse.mybir as mybir

    f32 = mybir.dt.float32
    f8 = mybir.dt.float8e4
    nc = bacc.Bacc(
        "TRN2",
        target_bir_lowering=False,
        debug=False,
        enable_asserts=False,
        num_devices=NCORES,
        # this kernel never reads the partition id (cores differ only by
        # their input slices) and uses no monotonic semaphores — dropping
        # both trims the framework preamble
        enable_partition_id=False,
        monotonic_sem_count=0,
    )

    # chunk-major DRAM layout: chunk i is one contiguous 266 KB block,
    # giving a single contiguous 2080 B M2S descriptor per partition
    x_in = nc.dram_tensor("x_in", (NCH, 128, 2 * DW), f8, kind="ExternalInput")
    sq_out = nc.dram_tensor("sq_out", (128, W), f32, kind="ExternalOutput")
    ST_out = nc.dram_tensor("ST_out", (CP, 2 * D), f32, kind="ExternalOutput")

    x_sb = nc.alloc_sbuf_tensor("x_sb", [128, W, DW], f8)
    ST_sb = nc.alloc_sbuf_tensor("ST_sb", [CP, 2 * D], f32)
    stats = nc.alloc_sbuf_tensor("stats", [128, W], f32)
    warm_sb = nc.alloc_sbuf_tensor("warm_sb", [128, 32], f8)

    pS = nc.alloc_psum_tensor("pS", [CP, D], f32)  # 2 banks
    pT = nc.alloc_psum_tensor("pT", [CP, D], f32)  # 2 banks
    # fp8 square scratch: keeps the scratch WRITES small (1 KB/partition/op)
    # so they do not stall the input-DMA SBUF writes; slot 0 = ACT, slot 1 =
    # DVE, reused serially per engine.  Only the f32 accum_out feeds the
    # result, the scratch value is never read.
    sq_scr = nc.alloc_sbuf_tensor("sq_scr", [128, 2, D], f8)

    k_sems = [nc.alloc_semaphore(f"k_sem{i}") for i in range(NCH)]
    warm_sem = nc.alloc_semaphore("warm_sem")
    sS = nc.alloc_semaphore("sS")
    sT = nc.alloc_semaphore("sT")
    ev = nc.alloc_semaphore("ev")
    sq_done = nc.alloc_semaphore("sq_done")
    out_sem = nc.alloc_semaphore("out_sem")

    Sq = mybir.ActivationFunctionType.Square
    Copy = mybir.ActivationFunctionType.Copy
    ADD = mybir.AluOpType.add
    MUL = mybir.AluOpType.mult
    DR = mybir.MatmulPerfMode.DoubleRow

    xs = x_sb.ap()

    def square(engine, w):
        # fused square + free-axis accumulate: stats[:, w] = sum_d x^2 over
        # tile w's 1024 data columns (one-hot tail excluded).
        if engine is nc.scalar:
            op = engine.activation(
                sq_scr.ap()[:, 0, :],
                xs[:, w, 0:D],
                Sq,
                accum_out=stats.ap()[:, w : w + 1],
            )
        else:
            op = engine.scalar_tensor_tensor(
                sq_scr.ap()[:, 1, :],
                xs[:, w, 0:D],
                0.0,
                xs[:, w, 0:D],
                ADD,
                MUL,
                accum_out=stats.ap()[:, w : w + 1],
            )
        op.then_inc(sq_done, 1)

    with nc.Block() as block:

        @block.sync
        def _(sync):
            # chunks 0,2 on the sync HWDGE ring; 1,3 on the scalar HWDGE
            # ring (issued there before the table load) — two rings drain
            # concurrently; chunk i = tiles (2i, 2i+1) = DR pair i
            for i in (0, 2):
                sync.dma_start(
                    x_sb.ap()[:, 2 * i : 2 * i + 2, :],
                    x_in.ap()[i, :, :],
                ).then_inc(k_sems[i], 16)
            # combined S|T output once both evacuations landed
            sync.wait_ge(ev, 2)
            sync.dma_start(ST_out.ap(), ST_sb.ap()).then_inc(out_sem, 16)
            # no explicit out_sem wait: the NEFF postamble's per-engine
            # DRAIN + runtime pending-DMA quiesce already order the output
            # DMAs before execution-complete
            sync.wait_ge(out_sem, 0)

        @block.scalar
        def _(scalar):
            for i in (1, 3):
                scalar.dma_start(
                    x_sb.ap()[:, 2 * i : 2 * i + 2, :],
                    x_in.ap()[i, :, :],
                ).then_inc(k_sems[i], 16)
            # ACT table load auto-inserts before the first ACTIVATE (the
            # single-wait square below), right after the issues above
            for w in SQ_ACT:
                scalar.wait_ge(k_sems[w // 2], 16)
                square(scalar, w)
            scalar.wait_ge(sS, 2)
            scalar.activation(ST_sb.ap()[:, 0:D], pS.ap(), Copy).then_inc(ev, 1)

        @block.vector
        def _(vector):
            for w in SQ_DVE:
                vector.wait_ge(k_sems[w // 2], 16)
                square(vector, w)
            vector.wait_ge(sT, 2)
            # f32 -> f32, no cast: the dtype-converting copy measured 1x
            vector.tensor_copy(ST_sb.ap()[:, D : 2 * D], pT.ap()).then_inc(ev, 1)

        @block.gpsimd
        def _(gpsimd):
            # sq_out rides the SW-DGE queue, keeping the HWDGE ring free
            gpsimd.wait_ge(sq_done, W)
            gpsimd.dma_start(sq_out.ap(), stats.ap()).then_inc(out_sem, 16)

        @block.tensor
        def _(tensor):
            # DoubleRow fp8: each matmul contracts a PAIR of 128-row k-tiles
            # (AP dim1 = pair index).  Accumulation groups per PSUM bank run
            # pairA (start) -> pairB (stop); chunk i holds exactly pair i.
            def mm(bank, dsem, pair, start, stop):
                a = 2 * pair
                lhsT = xs[:, a : a + 2, D:DW]
                for h in range(2):
                    m = tensor.matmul(
                        bank.ap()[:, 512 * h : 512 * (h + 1)],
                        lhsT,
                        xs[:, a : a + 2, 512 * h : 512 * (h + 1)],
                        start=start,
                        stop=stop,
                        perf_mode=DR,
                    )
                    if stop:
                        m.then_inc(dsem, 1)

            tensor.wait_ge(k_sems[0], 16)
            mm(pS, sS, 0, True, False)
            tensor.wait_ge(k_sems[1], 16)
            mm(pS, sS, 1, False, True)
            tensor.wait_ge(k_sems[2], 16)
            mm(pT, sT, 2, True, False)
            tensor.wait_ge(k_sems[3], 16)
            mm(pT, sT, 3, False, True)

    nc.compile()
    return nc


def _get_nc():
    if "nc" not in _STATE:
        _STATE["nc"] = build_nc_raw()
    return _STATE["nc"]


def kernel(fm_s, fm_t, targets, fusion_true=0, **_unused):
    global LAST_RUN
    import ml_dtypes
    from concourse.bass_utils import run_bass_kernel_spmd

    f8 = ml_dtypes.float8_e4m3
    fm_s = np.ascontiguousarray(np.asarray(fm_s, dtype=np.float32))
    fm_t = np.ascontiguousarray(np.asarray(fm_t, dtype=np.float32))
    tg = np.asarray(targets).astype(np.int64).ravel()
    assert fm_s.shape == (N, D) and fm_t.shape == (N, D) and tg.shape == (N,)

    oh = (tg[:, None] == np.arange(CP, dtype=np.int64)[None, :]).astype(np.float32)
    counts = np.bincount(tg, minlength=CP).astype(np.float64)[:CP]
    # append the one-hot columns to every row so each 128-row k-tile is
    # self-contained (the PE takes lhsT from the tile's own tail columns)
    s_aug = np.concatenate([fm_s, oh], axis=1).astype(f8)
    t_aug = np.concatenate([fm_t, oh], axis=1).astype(f8)

    in_maps = []
    for c in range(NCORES):
        # chunk-major + partition-major: chunk i holds tiles (2i, 2i+1);
        # x[i, p, :] = rows (256i + p, 256i + 128 + p) of this core's shard
        # (s tiles 0-3 then t tiles 4-7), each a contiguous 2080 B read.
        s_c = s_aug[c * RPC : (c + 1) * RPC].reshape(KT, 128, DW)
        t_c = t_aug[c * RPC : (c + 1) * RPC].reshape(KT, 128, DW)
        tiles = np.concatenate([s_c, t_c], axis=0)          # [W, 128, DW]
        x = np.ascontiguousarray(
            tiles.reshape(NCH, 2, 128, DW)
            .transpose(0, 2, 1, 3)
            .reshape(NCH, 128, 2 * DW)
        )
        in_maps.append({"x_in": x})

    nc = _get_nc()
    LAST_RUN = run_bass_kernel_spmd(nc, in_maps, list(range(NCORES)))
    res = LAST_RUN.results

    ss_tt = 0.0
    S = np.zeros((CP, D), np.float64)
    T = np.zeros((CP, D), np.float64)
    for r in res:
        ss_tt += float(r["sq_out"].astype(np.float64).sum())
        ST = r["ST_out"].astype(np.float64)
        S += ST[:, 0:D]
        T += ST[:, D : 2 * D]

    safe = np.where(counts > 0, counts, 1.0)
    dot = float(((S * T).sum(axis=1) / safe).sum())
    loss = (ss_tt - 2.0 * dot) / N
    return np.array(loss, dtype=np.float32)
